# revision 11
# baseline (speedup 1.0000x reference)
"""Trainium2 Bass kernel for causal multi-head attention with RoPE.

Sharding: tensor-parallel over heads. 16 heads / 8 cores = 2 heads per core.
Each core computes QKV projection for its 2 heads (full sequence), RoPE,
causal flash-style attention, and a partial output projection against its
slice of out_w columns; the host sums the 8 partial outputs.

v2: all matmuls in bf16 (FWL weight loads, so the PE is moving-operand
bound, not LDWEIGHTS bound), N=512 free dims everywhere, causal masking
applied as an extra accumulation matmul of a constant triangle into the
scores PSUM before exp (no gpsimd affine_select on the critical path),
diagonal score blocks narrowed, lag-2 software pipeline in attention, and
host-packed DRAM layouts so every DMA is 128 descriptors of >=4KB.
"""

import math
import os
import sys

for _p in ("/opt/trn_rl_repo",):
    if _p not in sys.path and os.path.isdir(_p):
        sys.path.insert(0, _p)

import numpy as np
import ml_dtypes

import concourse.bass as bass  # noqa: F401  (AP helpers)
import concourse.mybir as mybir
import concourse.tile as tile
from concourse import bacc
from concourse.bass_utils import run_bass_kernel_spmd

F32 = mybir.dt.float32
BF16 = mybir.dt.bfloat16
NPBF16 = ml_dtypes.bfloat16

B, T, C = 2, 2048, 2048
H, D = 16, 128
N_CORES = 8
HPC = H // N_CORES          # heads per core (2)
KC = C // 128               # 16 contraction blocks
TB = 512                    # token block (qkv + attention q-block)
NTB = T // TB               # 4 t-blocks per batch
NKB = T // 128              # 16 key blocks per batch
SCALE = 1.0 / math.sqrt(D)
MASKVAL = -1.0e9

_CACHED_NC = None


def build_nc():
    nc = bacc.Bacc("TRN2", target_bir_lowering=False)

    # DRAM inputs, host-packed so each DMA is contiguous per partition.
    xH = nc.dram_tensor("xH", [128, B, NTB, KC, TB], BF16, kind="ExternalInput")
    wqkH = nc.dram_tensor("wqkH", [128, 4, KC, 128], BF16, kind="ExternalInput")
    wvH = nc.dram_tensor("wvH", [128, KC, 256], BF16, kind="ExternalInput")
    owH = nc.dram_tensor("owH", [128, NTB, KC, TB], BF16, kind="ExternalInput")
    cosH = nc.dram_tensor("cosH", [128, T], BF16, kind="ExternalInput")
    sinH = nc.dram_tensor("sinH", [128, T], BF16, kind="ExternalInput")
    onesH = nc.dram_tensor("onesH", [128, 128], BF16, kind="ExternalInput")
    triH = nc.dram_tensor("triH", [128, 128], BF16, kind="ExternalInput")
    idH = nc.dram_tensor("idH", [128, 128], BF16, kind="ExternalInput")
    y = nc.dram_tensor("y", [B * HPC, 128, C], F32, kind="ExternalOutput")

    with tile.TileContext(nc) as tc:
        with tc.tile_pool(name="const", bufs=1) as cpool, \
             tc.tile_pool(name="xpool", bufs=2) as xpool, \
             tc.tile_pool(name="rotp", bufs=2) as rotp, \
             tc.tile_pool(name="vpool", bufs=2) as vpool, \
             tc.tile_pool(name="apool", bufs=4) as apool, \
             tc.tile_pool(name="tpool", bufs=2) as tpool, \
             tc.tile_pool(name="epool", bufs=4) as epool, \
             tc.tile_pool(name="rpool", bufs=2) as rpool, \
             tc.tile_pool(name="owpool", bufs=2) as owpool, \
             tc.tile_pool(name="ypool", bufs=2) as ypool, \
             tc.tile_pool(name="projps", bufs=2, space="PSUM") as projps, \
             tc.tile_pool(name="flowps", bufs=3, space="PSUM") as flowps, \
             tc.tile_pool(name="attps", bufs=2, space="PSUM") as attps, \
             tc.tile_pool(name="denps", bufs=1, space="PSUM") as denps:

            twqk = cpool.tile([128, 4, KC, 128], BF16)
            twv = cpool.tile([128, KC, 256], BF16)
            tcf = cpool.tile([128, T], BF16)
            tsn = cpool.tile([128, T], BF16)
            tones = cpool.tile([128, 128], BF16)
            ttri = cpool.tile([128, 128], BF16)
            tid = cpool.tile([128, 128], BF16)

            # x blocks go on the sync (SP) HWDGE ring; weights/constants on
            # the scalar (ACT) ring so the first QKV unit isn't queued
            # behind 5MB of constants.
            xgs = {}
            all_atns = {}

            def load_xg(b, tb):
                if (b, tb) not in xgs:
                    xg = xpool.tile([128, KC, TB], BF16, tag="xg",
                                    name=f"xg{b}_{tb}")
                    # halves so compute can start after the first arrives
                    nc.sync.dma_start(xg[:, 0:KC // 2], xH[:, b, tb, 0:KC // 2])
                    nc.sync.dma_start(xg[:, KC // 2:], xH[:, b, tb, KC // 2:])
                    xgs[(b, tb)] = xg
                return xgs[(b, tb)]

            load_xg(0, 0)
            nc.scalar.dma_start(twqk[:, 0], wqkH[:, 0])
            for m in range(1, 4):
                nc.scalar.dma_start(twqk[:, m], wqkH[:, m])
            nc.scalar.dma_start(twv[:], wvH[:, :, :])
            nc.scalar.dma_start(tcf[:], cosH[:, :])
            nc.scalar.dma_start(tsn[:], sinH[:, :])
            nc.scalar.dma_start(tones[:], onesH[:, :])
            nc.scalar.dma_start(ttri[:], triH[:, :])
            nc.scalar.dma_start(tid[:], idH[:, :])

            for b in range(B):
                # ---------------- QKV projection + RoPE ----------------
                rots = [rotp.tile([128, T], BF16, tag=f"rot{m}", name=f"rot{m}_{b}")
                        for m in range(4)]   # q_h0, q_h1, k_h0, k_h1
                vts = vpool.tile([128, NKB, 256], BF16, tag="vts", name=f"vts{b}")
                for tb in range(NTB):
                    ts_sl = slice(tb * TB, (tb + 1) * TB)
                    xg = load_xg(b, tb)
                    for m in range(4):
                        ps = projps.tile([128, TB], F32, tag="pp", name="psqk")
                        for k in range(KC):
                            nc.tensor.matmul(
                                ps[:], twqk[:, m, k], xg[:, k],
                                start=(k == 0), stop=(k == KC - 1))
                        # RoPE: rows 0:64 = x1, 64:128 = x2 of this head tensor
                        qsb = tpool.tile([128, TB], BF16, tag="qsb")
                        nc.scalar.copy(qsb[:], ps[:])
                        qsw = tpool.tile([128, TB], BF16, tag="qsw")
                        # gpsimd DIRECT2D: keeps the sync HWDGE ring free for
                        # x loads (a compute-dependent DMA in that FIFO would
                        # stall the next x block behind it)
                        nc.gpsimd.dma_start(qsw[0:64, :], qsb[64:128, :])
                        nc.gpsimd.dma_start(qsw[64:128, :], qsb[0:64, :])
                        pc = tpool.tile([128, TB], BF16, tag="pc")
                        nc.vector.tensor_mul(out=pc[:], in0=qsb[:],
                                             in1=tcf[:, ts_sl])
                        pn = tpool.tile([128, TB], BF16, tag="pn")
                        nc.gpsimd.tensor_mul(out=pn[:], in0=qsw[:],
                                             in1=tsn[:, ts_sl])
                        nc.vector.tensor_add(out=rots[m][:, ts_sl],
                                             in0=pc[:], in1=pn[:])
                    for vg in range(2):
                        psv = projps.tile([128, TB], F32, tag="pp", name="psv")
                        for half in range(2):
                            ts = 2 * vg + half
                            for k in range(KC):
                                nc.tensor.matmul(
                                    psv[:, half * 256:(half + 1) * 256],
                                    xg[:, k, ts * 128:(ts + 1) * 128],
                                    twv[:, k, :],
                                    start=(k == 0), stop=(k == KC - 1))
                        nc.vector.tensor_copy(
                            vts[:, 4 * tb + 2 * vg: 4 * tb + 2 * vg + 2, :],
                            psv[:])

                # prefetch next batch's first x blocks during this batch's
                # attention (xpool slots are free by now)
                if b + 1 < B:
                    load_xg(b + 1, 0)
                    load_xg(b + 1, 1)

                # ---------------- attention (lag-2 pipelined stream) -------
                atns = all_atns[b] = []
                for h in range(HPC):
                    rq, rk = rots[h], rots[2 + h]
                    atn = apool.tile([128, T], BF16, tag="atn",
                                     name=f"atn{b}_{h}")
                    atns.append(atn)
                    units = [(tb, si)
                             for tb in range(NTB)
                             for si in range(4 * (tb + 1))]
                    state = {}   # unit -> (et, lo, ps_att, ps_den)
                    cur = {}

                    def emit_front(u):
                        tb, si = u
                        r = si * 128 - tb * TB
                        lo = max(r, 0)
                        if si == 0:
                            cur['att'] = attps.tile([128, TB], F32, tag="psatt", name=f"psatt{b}_{h}_{tb}")
                            cur['den'] = denps.tile([128, TB], F32, tag="psden", name=f"psden{b}_{h}_{tb}")
                        sc = flowps.tile([128, TB], F32, tag="sc")
                        q_sl = slice(tb * TB + lo, (tb + 1) * TB)
                        diag = r >= 0
                        nc.tensor.matmul(
                            sc[:, lo:], rk[:, si * 128:(si + 1) * 128],
                            rq[:, q_sl], start=True, stop=(not diag))
                        if diag:
                            nc.tensor.matmul(
                                sc[:, r:r + 128], ttri[:], tid[:],
                                start=False, stop=True)
                        et = epool.tile([128, TB], BF16, tag="et")
                        nc.scalar.activation(
                            et[:, lo:], sc[:, lo:],
                            mybir.ActivationFunctionType.Exp, scale=SCALE)
                        state[u] = (et, lo, cur['att'], cur['den'])

                    def emit_back(u):
                        tb, si = u
                        et, lo, ps_att, ps_den = state.pop(u)
                        first = (si == 0)
                        last = (si == 4 * (tb + 1) - 1)
                        nc.tensor.matmul(
                            ps_den[:, lo:], tones[:], et[:, lo:],
                            start=first, stop=last)
                        nc.tensor.matmul(
                            ps_att[:, lo:],
                            vts[:, si, h * 128:(h + 1) * 128], et[:, lo:],
                            start=first, stop=last)
                        if last:
                            ts_sl = slice(tb * TB, (tb + 1) * TB)
                            rcp = rpool.tile([128, TB], F32, tag="rcp")
                            nc.vector.reciprocal_approx_fast(
                                out=rcp[:], in_=ps_den[:])
                            nc.vector.tensor_mul(
                                out=atn[:, ts_sl], in0=ps_att[:], in1=rcp[:])

                    LAG = 2
                    for i, u in enumerate(units):
                        emit_front(u)
                        if i >= LAG:
                            emit_back(units[i - LAG])
                    for u in units[-LAG:]:
                        emit_back(u)

            # -------- output projection (per-head 128-row slices) ----------
            # reference applies permute(0,2,1,3).reshape(B,T,C) to a
            # [B,T,H,D] tensor: out row t' = h*128 + t//16 uses head h,
            # col c' = (t%16)*128 + d.  Y_slice = attn_h.reshape(128,
            # 16*128) @ out_w.T, contracting over (u=t%16, d).
            # Hoisted after both batches so each out_w block loads once.
            for jb in range(NTB):
                owj = owpool.tile([128, KC, TB], BF16, tag="owj",
                                  name=f"owj{jb}")
                nc.scalar.dma_start(owj[:], owH[:, jb])
                for b in range(B):
                    for h in range(HPC):
                        av = all_atns[b][h][:].rearrange(
                            "p (a u) -> p a u", u=16)
                        psy = projps.tile([128, TB], F32, tag="pp", name="psy")
                        for u in range(KC):
                            nc.tensor.matmul(
                                psy[:], av[:, :, u], owj[:, u, :],
                                start=(u == 0), stop=(u == KC - 1))
                        ys = ypool.tile([128, TB], F32, tag="ys")
                        nc.scalar.copy(ys[:], psy[:])
                        nc.gpsimd.dma_start(
                            y[b * HPC + h, :, jb * TB:(jb + 1) * TB], ys[:])
    nc.compile()
    return nc


def _get_nc():
    global _CACHED_NC
    if _CACHED_NC is None:
        _CACHED_NC = build_nc()
    return _CACHED_NC


def _rope_tables():
    pos = np.arange(T, dtype=np.float64)[:, None]
    div = np.exp(np.arange(0, D, 2, dtype=np.float64) *
                 (-math.log(10000.0) / D))
    ang = pos * div  # [T, 64]
    sinT = np.sin(ang).T.astype(np.float32)  # [64, T]
    cosT = np.cos(ang).T.astype(np.float32)
    cosF = np.ascontiguousarray(np.concatenate([cosT, cosT], axis=0))
    sinS = np.ascontiguousarray(np.concatenate([-sinT, sinT], axis=0))
    return cosF.astype(NPBF16), sinS.astype(NPBF16)


def make_in_maps(x, qkv_w, out_w):
    # xH[p, b, tb, kb, t] = x[b, tb*TB+t, kb*128+p]
    xH = np.ascontiguousarray(
        x.reshape(B, NTB, TB, KC, 128).transpose(4, 0, 1, 3, 2)
    ).astype(NPBF16)
    # owH[p, jb, u, j] = out_w[jb*TB+j, u*128+p]
    owH = np.ascontiguousarray(
        out_w.reshape(NTB, TB, KC, 128).transpose(3, 0, 2, 1)
    ).astype(NPBF16)
    cosF, sinS = _rope_tables()
    ones = np.ones((128, 128), dtype=NPBF16)
    tri = np.zeros((128, 128), dtype=np.float32)
    for kk in range(128):
        tri[:kk, kk] = MASKVAL           # tri[p, kk] = MASKVAL where p < kk
    tri = tri.astype(NPBF16)
    ident = np.eye(128, dtype=np.float32).astype(NPBF16)
    in_maps = []
    for c in range(N_CORES):
        h0, h1 = 2 * c, 2 * c + 1
        wqk = np.stack([
            qkv_w[h0 * D:(h0 + 1) * D],
            qkv_w[h1 * D:(h1 + 1) * D],
            qkv_w[C + h0 * D:C + (h0 + 1) * D],
            qkv_w[C + h1 * D:C + (h1 + 1) * D],
        ], axis=0)                       # [4, 128, 2048]
        # wqkH[p, m, kb, d] = wqk[m, d, kb*128+p]
        wqkH = np.ascontiguousarray(
            wqk.reshape(4, 128, KC, 128).transpose(3, 0, 2, 1)
        ).astype(NPBF16)
        wv = np.concatenate([
            qkv_w[2 * C + h0 * D:2 * C + (h0 + 1) * D],
            qkv_w[2 * C + h1 * D:2 * C + (h1 + 1) * D],
        ], axis=0)                       # [256, 2048]
        # wvH[p, kb, dv] = wv[dv, kb*128+p]
        wvH = np.ascontiguousarray(
            wv.reshape(256, KC, 128).transpose(2, 1, 0)
        ).astype(NPBF16)
        in_maps.append({
            "xH": xH,
            "wqkH": wqkH,
            "wvH": wvH,
            "owH": owH,
            "cosH": cosF,
            "sinH": sinS,
            "onesH": ones,
            "triH": tri,
            "idH": ident,
        })
    return in_maps


def kernel(x, qkv_w, out_w, _trace=False, _trace_kwargs=None):
    x = np.asarray(x, dtype=np.float32)
    qkv_w = np.asarray(qkv_w, dtype=np.float32)
    out_w = np.asarray(out_w, dtype=np.float32)
    nc = _get_nc()
    in_maps = make_in_maps(x, qkv_w, out_w)
    kwargs = {}
    if _trace:
        kwargs["trace"] = True
        if _trace_kwargs:
            kwargs.update(_trace_kwargs)
    res = run_bass_kernel_spmd(nc, in_maps, core_ids=list(range(N_CORES)),
                               **kwargs)
    out = np.empty((B, T, C), dtype=np.float32)
    for c in range(N_CORES):
        yc = res.results[c]["y"]  # [B*HPC, 128, C]
        for b in range(B):
            for hl in range(HPC):
                hg = HPC * c + hl
                out[b, hg * 128:(hg + 1) * 128] = yc[b * HPC + hl]
    if _trace:
        return out, res
    return out
